# revision 22
# baseline (speedup 1.0000x reference)
"""Local contrast normalization (9x9 Gaussian) Trainium2 Bass kernel.

Input x: [64, 512, 512, 1] f32. Output same shape:
    mean = conv2d_same(x, g9x9)
    d    = x - mean
    s    = conv2d_same(d*d, g9x9)
    norm = sqrt(s); keep = norm > 0.5
    out  = where(keep, d / norm, d)

Pure data parallel: 8 images per core on 8 cores; each 512x512 image is
processed in 5 row-windows of 112 output rows (<=128 input rows incl.
the two 4-row conv halos).

Each 2D 9x9 conv is accumulated PE matmuls: the stationary operand is a
banded matrix carrying all 9 vertical taps, and the horizontal tap
offset is the rhs free-dim offset into a zero-margin-padded SBUF tile.
Horizontal taps (tap 0 carries 3.8e-4 of the kernel mass and is
dropped):
  - fp8e4m3 DoubleRow matmuls cover TWO taps each: the two DoubleRow
    planes are overlapping shifted views of the same tile, with the
    HW-required even plane stride (delta=4 pairs (1,5),(2,6),(3,7),(4,8)
    for conv2; (2,6),(3,7) for conv1).
  - conv1 keeps taps {1,8,4,5} in bf16 (taps 4,5 for accuracy — they
    carry 60% of the mass; 1,8 because fp8 pairing needs even-delta
    partners and the odd/even tap parity leaves them over); dj=4
    carries the folded +identity so PSUM directly holds d = x - mean.
fp8 weights are scaled by 16 (else the taps land subnormal); x is
pre-scaled by 1/16 host-side for conv1's fp8 taps, and conv2's 16x is
divided out inside the Ln activation's scale together with a global
compensation factor for the fp8 weight-quantization bias.

Tail (rsqrt-free; Rsqrt is blocked in bass and Ln/Exp/Square share one
activation table, explicitly preloaded to stop per-window table
reloads): u = ln(c/16*s + eps) [Act, bf16], rnorm = exp(-0.5u) [Act,
f32], mask = u > ln(0.25) [DVE ts bf16], t1 = (rnorm-1)*mask [DVE stt],
out = (t1+1)*d [DVE stt, reads d from PSUM, f32 out].  d^2 -> fp8 via
Act Square->bf16 then DVE tensor_scalar copy->fp8 (Act cannot write
fp8; DVE cannot read PSUM and write fp8).
"""

import sys

sys.path.insert(0, "/opt/trn_rl_repo")

import numpy as np

H = W = 512
IMGS_PER_CORE = 8
N_CORES = 8
CHUNK = 112  # output rows per window
SCL = 16.0   # fp8 weight scale (power of two)
LN_THR = float(np.log(0.25))

CONV1_BF16_TAPS = [1, 8, 4, 5]    # dj=4 carries the folded identity
CONV1_DR_PAIRS = [(2, 6), (3, 7)]
CONV2_DR_PAIRS = [(1, 5), (2, 6), (3, 7), (4, 8)]

# square: Act Square psum->bf16, then DVE copy bf16->fp8 (DVE cannot
# read PSUM and write fp8 in one op — walrus rejects it)


def _gauss2d():
    # replicate reference._gauss_kernel exactly
    sigmah = 9 / 6.0
    ii = np.arange(9, dtype=np.float64)
    r2 = (ii[:, None] - 4.5) ** 2 + (ii[None, :] - 4.5) ** 2
    g = np.exp(-r2 / (2.0 * sigmah)).astype(np.float32)
    g = g / g.sum()
    return g  # [9(dv), 9(dj)]


def _windows():
    out = []
    for c in range((H + CHUNK - 1) // CHUNK):
        O0, O1 = CHUNK * c, min(CHUNK * c + CHUNK, H)
        D0, D1 = max(0, O0 - 4), min(H, O1 + 4)
        X0, X1 = max(0, D0 - 4), min(H, D1 + 4)
        out.append((O0, O1, D0, D1, X0, X1))
    return out


WINDOWS = _windows()
N_WIN = len(WINDOWS)
WTYPE_OF = [0 if c == 0 else 1 for c in range(N_WIN)]


def _quant_f8(a):
    import ml_dtypes

    return np.asarray(a, np.float32).astype(ml_dtypes.float8_e4m3)


def _band(g, dj, xd, scale, ident=False):
    A = np.zeros((128, 128), np.float32)
    for k in range(128):
        for m in range(128):
            dv = k - m - xd + 4
            v = 0.0
            if 0 <= dv <= 8:
                v = -scale * g[dv, dj]
            if ident and k - m == xd:
                v += 1.0
            A[k, m] = v
    return A


def _gen_weights():
    """w1b [128, 2(vt)*4, 128] bf16-ready f32: slots (vt, taps 1,8,4+Id,5).
       w18 [128, 2(vt)*2(pair)*2(plane), 128] fp8: conv1 -16g bands.
       w28 [128, 4(pair)*2(plane), 128] fp8: conv2 +16g bands.
       comp: compensation for fp8 weight quantization of conv2."""
    g = _gauss2d()
    w1b = np.zeros((128, 8, 128), np.float32)
    w18 = np.zeros((128, 8, 128), np.float32)
    for vt, xd in enumerate([0, 4]):
        for t, dj in enumerate(CONV1_BF16_TAPS):
            w1b[:, vt * 4 + t, :] = _band(g, dj, xd, 1.0, ident=(dj == 4))
        for pi, pair in enumerate(CONV1_DR_PAIRS):
            for i, dj in enumerate(pair):
                w18[:, (vt * 2 + pi) * 2 + i, :] = -_band(g, dj, xd, -SCL)
    w28 = np.zeros((128, 8, 128), np.float32)
    for pi, pair in enumerate(CONV2_DR_PAIRS):
        for i, dj in enumerate(pair):
            w28[:, pi * 2 + i, :] = -_band(g, dj, 0, SCL)

    used = [dj for pair in CONV2_DR_PAIRS for dj in pair]
    sum_g = float(sum(g[dv, dj] for dv in range(9) for dj in used))
    sum_q = float(
        sum(
            _quant_f8(SCL * g[dv, dj]).astype(np.float32) / SCL
            for dv in range(9)
            for dj in used
        )
    )
    comp = sum_g / sum_q
    return w1b, _quant_f8(w18), _quant_f8(w28), comp


def _two_plane(bass_mod, tile_ap, n_part, dj_lo, dj_hi):
    """AP [K, 2, 512] over a [128, 520] margin tile: plane i at free
    offset dj_i (overlapping shifted views; stride dj_hi-dj_lo must be
    even for DoubleRow)."""
    base = tile_ap[0:n_part, dj_lo : dj_lo + 512]
    ap_list = [list(p) for p in base.ap]
    new_ap = [ap_list[0], [dj_hi - dj_lo, 2], ap_list[1]]
    return bass_mod.AP(tensor=base.tensor, offset=base.offset, ap=new_ap)


def _build_program():
    import concourse.bass as bass
    import concourse.bacc as bacc
    import concourse.tile as tile
    from concourse import mybir

    f32 = mybir.dt.float32
    bf16 = mybir.dt.bfloat16
    f8 = mybir.dt.float8e4
    DR = mybir.MatmulPerfMode.DoubleRow
    A = mybir.AluOpType
    AF = mybir.ActivationFunctionType

    _, _, _, comp = _gen_weights()

    nc = bacc.Bacc("TRN2", target_bir_lowering=False, debug=False,
                   num_devices=N_CORES)

    rows = IMGS_PER_CORE * H
    x_dram = nc.dram_tensor("x", [rows, W], bf16, kind="ExternalInput")
    x8_dram = nc.dram_tensor("x8", [rows, W], f8, kind="ExternalInput")
    w1b_dram = nc.dram_tensor("w1b", [128, 8 * 128], bf16, kind="ExternalInput")
    w18_dram = nc.dram_tensor("w18", [128, 8 * 128], f8, kind="ExternalInput")
    w28_dram = nc.dram_tensor("w28", [128, 8 * 128], f8, kind="ExternalInput")
    o_dram = nc.dram_tensor("out", [rows, W], f32, kind="ExternalOutput")

    with tile.TileContext(nc) as tc:
        with (
            tc.tile_pool(name="wpool", bufs=1) as wpool,
            tc.tile_pool(name="xpool", bufs=8) as xpool,
            tc.tile_pool(name="x8pool", bufs=8) as x8pool,
            tc.tile_pool(name="dsqpool", bufs=8) as dsqpool,
            tc.tile_pool(name="spool", bufs=8) as spool,
            tc.tile_pool(name="opool", bufs=8) as opool,
            tc.tile_pool(name="ps1", bufs=6, space=bass.MemorySpace.PSUM) as ps1,
            tc.tile_pool(name="ps2", bufs=2, space=bass.MemorySpace.PSUM) as ps2,
        ):
            w1b_sb = wpool.tile([128, 8, 128], bf16)
            w18_sb = wpool.tile([128, 8, 128], f8)
            w28_sb = wpool.tile([128, 8, 128], f8)
            nc.sync.dma_start(w1b_sb[:].rearrange("k v m -> k (v m)"), w1b_dram.ap())
            nc.sync.dma_start(w18_sb[:].rearrange("k v m -> k (v m)"), w18_dram.ap())
            nc.sync.dma_start(w28_sb[:].rearrange("k v m -> k (v m)"), w28_dram.ap())
            eps_sb = wpool.tile([128, 1], f32)
            nc.vector.memset(eps_sb[:], 1e-12)
            from concourse.hw_specs import get_activation_tables
            _tabs = get_activation_tables(nc.m.arch)
            _tid = next(i for i, (_nm, _s) in enumerate(_tabs.items())
                        if AF.Ln in _s and AF.Exp in _s and AF.Square in _s)
            nc.scalar.add_instruction(mybir.InstLoadActFuncSet(
                name=f"I-{nc.next_id()}", act_func_set_id=_tid, ins=[], outs=[]))

            widx = 0
            for i in range(IMGS_PER_CORE):
                for c in range(N_WIN):
                    O0, O1, D0, D1, X0, X1 = WINDOWS[c]
                    nX, nD, nO = X1 - X0, D1 - D0, O1 - O0
                    off2 = O0 - D0
                    vt = WTYPE_OF[c]
                    R = slice(0, nD)

                    x_win = xpool.tile([128, 520], bf16, tag="xwin")
                    if widx < 8:
                        nc.gpsimd.memset(x_win[0:128, 0:4], 0.0)
                        nc.gpsimd.memset(x_win[0:128, 516:520], 0.0)
                    nc.sync.dma_start(
                        x_win[0:nX, 4:516],
                        x_dram.ap()[i * H + X0 : i * H + X1, :],
                    )
                    x8_win = x8pool.tile([128, 520], f8, tag="x8win")
                    if widx < 8:
                        nc.gpsimd.memset(x8_win[0:128, 0:4], 0.0)
                        nc.gpsimd.memset(x8_win[0:128, 516:520], 0.0)
                    nc.sync.dma_start(
                        x8_win[0:nX, 4:516],
                        x8_dram.ap()[i * H + X0 : i * H + X1, :],
                    )

                    # conv1 -> psum1 = d  (identity folded into tap 4)
                    psum1 = ps1.tile([128, 512], f32, tag="d")
                    n1 = len(CONV1_DR_PAIRS) + len(CONV1_BF16_TAPS)
                    mi = 0
                    for pi, (lo, hi) in enumerate(CONV1_DR_PAIRS):
                        j = (vt * 2 + pi) * 2
                        nc.tensor.matmul(
                            psum1[0:nD, :],
                            w18_sb[0:nX, j : j + 2, 0:nD],
                            _two_plane(bass, x8_win, nX, lo, hi),
                            start=(mi == 0), stop=(mi == n1 - 1),
                            perf_mode=DR,
                        )
                        mi += 1
                    for t, dj in enumerate(CONV1_BF16_TAPS):
                        nc.tensor.matmul(
                            psum1[0:nD, :],
                            w1b_sb[0:nX, vt * 4 + t, 0:nD],
                            x_win[0:nX, dj : dj + 512],
                            start=(mi == 0), stop=(mi == n1 - 1),
                        )
                        mi += 1

                    # d^2 -> fp8 dsq tile (margins zeroed)
                    dsq = dsqpool.tile([128, 520], f8, tag="dsq")
                    if widx < 8:
                        nc.gpsimd.memset(dsq[0:128, 0:4], 0.0)
                        nc.gpsimd.memset(dsq[0:128, 516:520], 0.0)
                    dsqb = spool.tile([128, 512], bf16, tag="dsqb")
                    nc.scalar.activation(dsqb[R, :], psum1[R, :], AF.Square)
                    nc.vector.tensor_scalar(
                        dsq[0:nD, 4:516], dsqb[R, :], 0.0, None, A.add
                    )

                    # conv2 -> psum2 = 16 * conv(d^2, g)
                    psum2 = ps2.tile([128, 512], f32, tag="s")
                    for pi, (lo, hi) in enumerate(CONV2_DR_PAIRS):
                        j = pi * 2
                        nc.tensor.matmul(
                            psum2[0:nD, :],
                            w28_sb[0:nD, j : j + 2, 0:nD],
                            _two_plane(bass, dsq, nD, lo, hi),
                            start=(pi == 0),
                            stop=(pi == len(CONV2_DR_PAIRS) - 1),
                            perf_mode=DR,
                        )

                    # u = ln(comp/16 * s' + eps);  rnorm = exp(-u/2)
                    u = spool.tile([128, 512], bf16, tag="u")
                    nc.scalar.activation(
                        u[R, :], psum2[R, :], AF.Ln,
                        bias=eps_sb[R, :], scale=float(comp / SCL),
                    )
                    rnorm = spool.tile([128, 512], f32, tag="rnorm")
                    nc.scalar.activation(rnorm[R, :], u[R, :], AF.Exp, scale=-0.5)
                    mask = spool.tile([128, 512], bf16, tag="mask")
                    nc.vector.tensor_scalar(
                        mask[R, :], u[R, :], LN_THR, None, A.is_gt,
                    )
                    t1 = spool.tile([128, 512], f32, tag="t1")
                    nc.vector.scalar_tensor_tensor(
                        t1[R, :], rnorm[R, :], -1.0, mask[R, :], A.add, A.mult,
                    )
                    outt = opool.tile([128, 512], f32, tag="out")
                    nc.vector.scalar_tensor_tensor(
                        outt[R, :], t1[R, :], 1.0, psum1[R, :], A.add, A.mult,
                    )
                    nc.gpsimd.dma_start(
                        o_dram.ap()[i * H + O0 : i * H + O1, :],
                        outt[off2 : off2 + nO, :],
                    )
                    widx += 1

    nc.compile()
    return nc


_NC = None


def _get_nc():
    global _NC
    if _NC is None:
        _NC = _build_program()
    return _NC


def _run(x_full, trace=False, **kw):
    from concourse import bass_utils

    nc = _get_nc()
    w1b, w18, w28, _ = _gen_weights()
    import ml_dtypes

    bf = ml_dtypes.bfloat16
    f8 = ml_dtypes.float8_e4m3
    x_full = np.asarray(x_full, dtype=np.float32).reshape(64, H, W)
    w1b_h = np.ascontiguousarray(w1b.reshape(128, 8 * 128)).astype(bf)
    w18_h = np.ascontiguousarray(w18.reshape(128, 8 * 128))
    w28_h = np.ascontiguousarray(w28.reshape(128, 8 * 128))
    in_maps = []
    for core in range(N_CORES):
        shard = np.ascontiguousarray(
            x_full[core * IMGS_PER_CORE : (core + 1) * IMGS_PER_CORE].reshape(
                IMGS_PER_CORE * H, W
            )
        )
        in_maps.append({
            "x": shard.astype(bf),
            "x8": (shard / SCL).astype(f8),
            "w1b": w1b_h,
            "w18": w18_h,
            "w28": w28_h,
        })
    res = bass_utils.run_bass_kernel_spmd(
        nc, in_maps, core_ids=list(range(N_CORES)), trace=trace, **kw
    )
    out = np.concatenate(
        [r["out"].astype(np.float32).reshape(IMGS_PER_CORE, H, W)
         for r in res.results],
        axis=0,
    )
    return out.reshape(64, H, W, 1), res


def kernel(x):
    out, _ = _run(x)
    return out


# revision 23
# speedup vs baseline: 1.0116x; 1.0116x over previous
"""Local contrast normalization (9x9 Gaussian) Trainium2 Bass kernel.

Input x: [64, 512, 512, 1] f32. Output same shape:
    mean = conv2d_same(x, g9x9)
    d    = x - mean
    s    = conv2d_same(d*d, g9x9)
    norm = sqrt(s); keep = norm > 0.5
    out  = where(keep, d / norm, d)

Pure data parallel: 8 images per core on 8 cores; each 512x512 image is
processed in 5 row-windows of 112 output rows (<=128 input rows incl.
the two 4-row conv halos).

Each 2D 9x9 conv is accumulated PE matmuls: the stationary operand is a
banded matrix carrying all 9 vertical taps, and the horizontal tap
offset is the rhs free-dim offset into a zero-margin-padded SBUF tile.
Horizontal taps (tap 0 carries 3.8e-4 of the kernel mass and is
dropped):
  - fp8e4m3 DoubleRow matmuls cover TWO taps each: the two DoubleRow
    planes are overlapping shifted views of the same tile, with the
    HW-required even plane stride (delta=4 pairs (1,5),(2,6),(3,7),(4,8)
    for conv2; (2,6),(3,7) for conv1).
  - conv1 keeps taps {1,8,4,5} in bf16 (taps 4,5 for accuracy — they
    carry 60% of the mass; 1,8 because fp8 pairing needs even-delta
    partners and the odd/even tap parity leaves them over); dj=4
    carries the folded +identity so PSUM directly holds d = x - mean.
fp8 weights are scaled by 16 (else the taps land subnormal); x is
pre-scaled by 1/16 host-side for conv1's fp8 taps, and conv2's 16x is
divided out inside the Ln activation's scale together with a global
compensation factor for the fp8 weight-quantization bias.

Tail (rsqrt-free; Rsqrt is blocked in bass and Ln/Exp/Square share one
activation table, explicitly preloaded to stop per-window table
reloads): u = ln(c/16*s + eps) [Act, bf16], rnorm = exp(-0.5u) [Act,
f32], mask = u > ln(0.25) [DVE ts bf16], t1 = (rnorm-1)*mask [DVE stt],
out = (t1+1)*d [DVE stt, reads d from PSUM, f32 out].  d^2 -> fp8 via
Act Square->bf16 then DVE tensor_scalar copy->fp8 (Act cannot write
fp8; DVE cannot read PSUM and write fp8).
"""

import sys

sys.path.insert(0, "/opt/trn_rl_repo")

import numpy as np

H = W = 512
IMGS_PER_CORE = 8
N_CORES = 8
CHUNK = 112  # output rows per window
SCL = 16.0   # fp8 weight scale (power of two)
LN_THR = float(np.log(0.25))

CONV1_BF16_TAPS = [1, 8, 4, 5]    # dj=4 carries the folded identity
CONV1_DR_PAIRS = [(2, 6), (3, 7)]
CONV2_DR_PAIRS = [(1, 5), (2, 6), (3, 7), (4, 8)]

# square: Act Square psum->bf16, then DVE copy bf16->fp8 (DVE cannot
# read PSUM and write fp8 in one op — walrus rejects it)


def _gauss2d():
    # replicate reference._gauss_kernel exactly
    sigmah = 9 / 6.0
    ii = np.arange(9, dtype=np.float64)
    r2 = (ii[:, None] - 4.5) ** 2 + (ii[None, :] - 4.5) ** 2
    g = np.exp(-r2 / (2.0 * sigmah)).astype(np.float32)
    g = g / g.sum()
    return g  # [9(dv), 9(dj)]


def _windows():
    out = []
    for c in range((H + CHUNK - 1) // CHUNK):
        O0, O1 = CHUNK * c, min(CHUNK * c + CHUNK, H)
        D0, D1 = max(0, O0 - 4), min(H, O1 + 4)
        X0, X1 = max(0, D0 - 4), min(H, D1 + 4)
        out.append((O0, O1, D0, D1, X0, X1))
    return out


WINDOWS = _windows()
N_WIN = len(WINDOWS)
WTYPE_OF = [0 if c == 0 else 1 for c in range(N_WIN)]


def _quant_f8(a):
    import ml_dtypes

    return np.asarray(a, np.float32).astype(ml_dtypes.float8_e4m3)


def _band(g, dj, xd, scale, ident=False):
    A = np.zeros((128, 128), np.float32)
    for k in range(128):
        for m in range(128):
            dv = k - m - xd + 4
            v = 0.0
            if 0 <= dv <= 8:
                v = -scale * g[dv, dj]
            if ident and k - m == xd:
                v += 1.0
            A[k, m] = v
    return A


def _gen_weights():
    """w1b [128, 2(vt)*4, 128] bf16-ready f32: slots (vt, taps 1,8,4+Id,5).
       w18 [128, 2(vt)*2(pair)*2(plane), 128] fp8: conv1 -16g bands.
       w28 [128, 4(pair)*2(plane), 128] fp8: conv2 +16g bands.
       comp: compensation for fp8 weight quantization of conv2."""
    g = _gauss2d()
    w1b = np.zeros((128, 8, 128), np.float32)
    w18 = np.zeros((128, 8, 128), np.float32)
    for vt, xd in enumerate([0, 4]):
        for t, dj in enumerate(CONV1_BF16_TAPS):
            w1b[:, vt * 4 + t, :] = _band(g, dj, xd, 1.0, ident=(dj == 4))
        for pi, pair in enumerate(CONV1_DR_PAIRS):
            for i, dj in enumerate(pair):
                w18[:, (vt * 2 + pi) * 2 + i, :] = -_band(g, dj, xd, -SCL)
    w28 = np.zeros((128, 8, 128), np.float32)
    for pi, pair in enumerate(CONV2_DR_PAIRS):
        for i, dj in enumerate(pair):
            w28[:, pi * 2 + i, :] = -_band(g, dj, 0, SCL)

    used = [dj for pair in CONV2_DR_PAIRS for dj in pair]
    sum_g = float(sum(g[dv, dj] for dv in range(9) for dj in used))
    sum_q = float(
        sum(
            _quant_f8(SCL * g[dv, dj]).astype(np.float32) / SCL
            for dv in range(9)
            for dj in used
        )
    )
    comp = sum_g / sum_q
    return w1b, _quant_f8(w18), _quant_f8(w28), comp


def _two_plane(bass_mod, tile_ap, n_part, dj_lo, dj_hi):
    """AP [K, 2, 512] over a [128, 520] margin tile: plane i at free
    offset dj_i (overlapping shifted views; stride dj_hi-dj_lo must be
    even for DoubleRow)."""
    base = tile_ap[0:n_part, dj_lo : dj_lo + 512]
    ap_list = [list(p) for p in base.ap]
    new_ap = [ap_list[0], [dj_hi - dj_lo, 2], ap_list[1]]
    return bass_mod.AP(tensor=base.tensor, offset=base.offset, ap=new_ap)


def _build_program():
    import concourse.bass as bass
    import concourse.bacc as bacc
    import concourse.tile as tile
    from concourse import mybir

    f32 = mybir.dt.float32
    bf16 = mybir.dt.bfloat16
    f8 = mybir.dt.float8e4
    DR = mybir.MatmulPerfMode.DoubleRow
    A = mybir.AluOpType
    AF = mybir.ActivationFunctionType

    _, _, _, comp = _gen_weights()

    nc = bacc.Bacc("TRN2", target_bir_lowering=False, debug=False,
                   num_devices=N_CORES)

    rows = IMGS_PER_CORE * H
    x_dram = nc.dram_tensor("x", [rows, W], bf16, kind="ExternalInput")
    x8_dram = nc.dram_tensor("x8", [rows, W], f8, kind="ExternalInput")
    w1b_dram = nc.dram_tensor("w1b", [128, 8 * 128], bf16, kind="ExternalInput")
    w18_dram = nc.dram_tensor("w18", [128, 8 * 128], f8, kind="ExternalInput")
    w28_dram = nc.dram_tensor("w28", [128, 8 * 128], f8, kind="ExternalInput")
    o_dram = nc.dram_tensor("out", [rows, W], f32, kind="ExternalOutput")

    with tile.TileContext(nc) as tc:
        with (
            tc.tile_pool(name="wpool", bufs=1) as wpool,
            tc.tile_pool(name="xpool", bufs=6) as xpool,
            tc.tile_pool(name="x8pool", bufs=6) as x8pool,
            tc.tile_pool(name="dsqpool", bufs=6) as dsqpool,
            tc.tile_pool(name="spool", bufs=6) as spool,
            tc.tile_pool(name="opool", bufs=6) as opool,
            tc.tile_pool(name="ps1", bufs=6, space=bass.MemorySpace.PSUM) as ps1,
            tc.tile_pool(name="ps2", bufs=2, space=bass.MemorySpace.PSUM) as ps2,
        ):
            w1b_sb = wpool.tile([128, 8, 128], bf16)
            w18_sb = wpool.tile([128, 8, 128], f8)
            w28_sb = wpool.tile([128, 8, 128], f8)
            nc.sync.dma_start(w1b_sb[:].rearrange("k v m -> k (v m)"), w1b_dram.ap())
            nc.sync.dma_start(w18_sb[:].rearrange("k v m -> k (v m)"), w18_dram.ap())
            nc.sync.dma_start(w28_sb[:].rearrange("k v m -> k (v m)"), w28_dram.ap())
            eps_sb = wpool.tile([128, 1], f32)
            nc.vector.memset(eps_sb[:], 1e-12)
            from concourse.hw_specs import get_activation_tables
            _tabs = get_activation_tables(nc.m.arch)
            _tid = next(i for i, (_nm, _s) in enumerate(_tabs.items())
                        if AF.Ln in _s and AF.Exp in _s and AF.Square in _s)
            nc.scalar.add_instruction(mybir.InstLoadActFuncSet(
                name=f"I-{nc.next_id()}", act_func_set_id=_tid, ins=[], outs=[]))

            widx = 0
            for i in range(IMGS_PER_CORE):
                for c in range(N_WIN):
                    O0, O1, D0, D1, X0, X1 = WINDOWS[c]
                    nX, nD, nO = X1 - X0, D1 - D0, O1 - O0
                    off2 = O0 - D0
                    vt = WTYPE_OF[c]
                    R = slice(0, nD)

                    x_win = xpool.tile([128, 520], bf16, tag="xwin")
                    if widx < 6:
                        nc.gpsimd.memset(x_win[0:128, 0:4], 0.0)
                        nc.gpsimd.memset(x_win[0:128, 516:520], 0.0)
                    nc.sync.dma_start(
                        x_win[0:nX, 4:516],
                        x_dram.ap()[i * H + X0 : i * H + X1, :],
                    )
                    x8_win = x8pool.tile([128, 520], f8, tag="x8win")
                    if widx < 6:
                        nc.gpsimd.memset(x8_win[0:128, 0:4], 0.0)
                        nc.gpsimd.memset(x8_win[0:128, 516:520], 0.0)
                    nc.sync.dma_start(
                        x8_win[0:nX, 4:516],
                        x8_dram.ap()[i * H + X0 : i * H + X1, :],
                    )

                    # conv1 -> psum1 = d  (identity folded into tap 4)
                    psum1 = ps1.tile([128, 512], f32, tag="d")
                    n1 = len(CONV1_DR_PAIRS) + len(CONV1_BF16_TAPS)
                    mi = 0
                    for pi, (lo, hi) in enumerate(CONV1_DR_PAIRS):
                        j = (vt * 2 + pi) * 2
                        nc.tensor.matmul(
                            psum1[0:nD, :],
                            w18_sb[0:nX, j : j + 2, 0:nD],
                            _two_plane(bass, x8_win, nX, lo, hi),
                            start=(mi == 0), stop=(mi == n1 - 1),
                            perf_mode=DR,
                        )
                        mi += 1
                    for t, dj in enumerate(CONV1_BF16_TAPS):
                        nc.tensor.matmul(
                            psum1[0:nD, :],
                            w1b_sb[0:nX, vt * 4 + t, 0:nD],
                            x_win[0:nX, dj : dj + 512],
                            start=(mi == 0), stop=(mi == n1 - 1),
                        )
                        mi += 1

                    # d^2 -> fp8 dsq tile (margins zeroed)
                    dsq = dsqpool.tile([128, 520], f8, tag="dsq")
                    if widx < 6:
                        nc.gpsimd.memset(dsq[0:128, 0:4], 0.0)
                        nc.gpsimd.memset(dsq[0:128, 516:520], 0.0)
                    dsqb = spool.tile([128, 512], bf16, tag="dsqb")
                    nc.scalar.activation(dsqb[R, :], psum1[R, :], AF.Square)
                    nc.vector.tensor_scalar(
                        dsq[0:nD, 4:516], dsqb[R, :], 0.0, None, A.add
                    )

                    # conv2 -> psum2 = 16 * conv(d^2, g)
                    psum2 = ps2.tile([128, 512], f32, tag="s")
                    for pi, (lo, hi) in enumerate(CONV2_DR_PAIRS):
                        j = pi * 2
                        nc.tensor.matmul(
                            psum2[0:nD, :],
                            w28_sb[0:nD, j : j + 2, 0:nD],
                            _two_plane(bass, dsq, nD, lo, hi),
                            start=(pi == 0),
                            stop=(pi == len(CONV2_DR_PAIRS) - 1),
                            perf_mode=DR,
                        )

                    # u = ln(comp/16 * s' + eps);  rnorm = exp(-u/2)
                    u = spool.tile([128, 512], bf16, tag="u")
                    nc.scalar.activation(
                        u[R, :], psum2[R, :], AF.Ln,
                        bias=eps_sb[R, :], scale=float(comp / SCL),
                    )
                    rnorm = spool.tile([128, 512], f32, tag="rnorm")
                    nc.scalar.activation(rnorm[R, :], u[R, :], AF.Exp, scale=-0.5)
                    mask = spool.tile([128, 512], bf16, tag="mask")
                    nc.vector.tensor_scalar(
                        mask[R, :], u[R, :], LN_THR, None, A.is_gt,
                    )
                    t1 = spool.tile([128, 512], f32, tag="t1")
                    nc.vector.scalar_tensor_tensor(
                        t1[R, :], rnorm[R, :], -1.0, mask[R, :], A.add, A.mult,
                    )
                    outt = opool.tile([128, 512], f32, tag="out")
                    nc.vector.scalar_tensor_tensor(
                        outt[R, :], t1[R, :], 1.0, psum1[R, :], A.add, A.mult,
                    )
                    nc.gpsimd.dma_start(
                        o_dram.ap()[i * H + O0 : i * H + O1, :],
                        outt[off2 : off2 + nO, :],
                    )
                    widx += 1

    nc.compile()
    return nc


_NC = None


def _get_nc():
    global _NC
    if _NC is None:
        _NC = _build_program()
    return _NC


def _run(x_full, trace=False, **kw):
    from concourse import bass_utils

    nc = _get_nc()
    w1b, w18, w28, _ = _gen_weights()
    import ml_dtypes

    bf = ml_dtypes.bfloat16
    f8 = ml_dtypes.float8_e4m3
    x_full = np.asarray(x_full, dtype=np.float32).reshape(64, H, W)
    w1b_h = np.ascontiguousarray(w1b.reshape(128, 8 * 128)).astype(bf)
    w18_h = np.ascontiguousarray(w18.reshape(128, 8 * 128))
    w28_h = np.ascontiguousarray(w28.reshape(128, 8 * 128))
    in_maps = []
    for core in range(N_CORES):
        shard = np.ascontiguousarray(
            x_full[core * IMGS_PER_CORE : (core + 1) * IMGS_PER_CORE].reshape(
                IMGS_PER_CORE * H, W
            )
        )
        in_maps.append({
            "x": shard.astype(bf),
            "x8": (shard / SCL).astype(f8),
            "w1b": w1b_h,
            "w18": w18_h,
            "w28": w28_h,
        })
    res = bass_utils.run_bass_kernel_spmd(
        nc, in_maps, core_ids=list(range(N_CORES)), trace=trace, **kw
    )
    out = np.concatenate(
        [r["out"].astype(np.float32).reshape(IMGS_PER_CORE, H, W)
         for r in res.results],
        axis=0,
    )
    return out.reshape(64, H, W, 1), res


def kernel(x):
    out, _ = _run(x)
    return out
